# revision 16
# baseline (speedup 1.0000x reference)
"""Trainium2 Bass kernel for ConvTemporalGraphical (gnn_message_passing).

Reference computation (fp32):
    y   = einsum('nctv,oc->notv', x, W) + b        # 1x1 conv channel mix
    out = einsum('nkctv,kvw->nctw', y.reshape(n,K,C,t,v), A)

Shapes: x [16,128,256,64] f32, A [3,64,64], W [384,128], b [384].

Strategy (8 NeuronCores, data-parallel over N, 2 samples per core):
  W-contraction first, producing the intermediate TRANSPOSED so no PE
  transposes are needed anywhere:
      stage 1:  yT[(t,v), (k,c)] = sum_ci x[ci,t,v] * Wt[ci,(k,c)]
                (lhsT = x in its natural layout, fp32r, FD=384)
      stage 2:  out[c, (t,w)]   += sum_{v} yT[(t,v),(k,c)] * A[k,v,w]
                (bf16, FD=128, PSUM-accumulated over k; MA_k block-diag
                 over the two t's of a pair to use all 128 partitions)

  Engine budget: PSUM can only be drained by DVE and ACT (not GpSimd,
  not DMA), and only DVE can run TensorTensor. So both PSUM drains are
  PLAIN casts/copies, pinned to alternate between DVE and ACT, and the
  bias2[c,w] = sum_{k,v} b[(k,c)] A[k,v,w] add runs on the otherwise-
  idle GpSimd engine in SBUF (out_sb + bias -> out_sb2) before the
  output DMA. Output copies and the bias add are batched per 2 groups.

  Measured pacing (this hw): FD=384 f32r b2b 195ns/MM, FD=128 bf16 b2b
  62ns/MM -> PE ~1525ns per 8-t group = ~98us/core (the wall). DVE
  ~1280ns/group, ACT ~1125, GpSimd ~1380, SP ~613.

kernel(**inputs) shards on host, runs the SPMD program on cores 0-7, and
concatenates the per-core outputs.
"""

import numpy as np
import ml_dtypes

import concourse.bass as bass
import concourse.mybir as mybir
from concourse import bacc
from concourse.bass_utils import run_bass_kernel_spmd
from concourse.tile import TileContext

F32 = mybir.dt.float32
F32R = mybir.dt.float32r
BF16 = mybir.dt.bfloat16

N, C_IN, C_OUT, K, T, V = 16, 128, 128, 3, 256, 64
N_CORES = 8
N_PER_CORE = N // N_CORES  # 2
G = 8                      # t's per group
N_GROUPS = T // G          # 32 groups per sample


def build(reps: int = 1):
    nc = bacc.Bacc(
        "TRN2", target_bir_lowering=False, debug=False, num_devices=N_CORES
    )
    xs = nc.dram_tensor("xs", [N_PER_CORE, C_IN, T, V], F32, kind="ExternalInput")
    wt = nc.dram_tensor("wt", [C_IN, K * C_OUT], F32, kind="ExternalInput")
    mak = nc.dram_tensor("mak", [128, K, 128], BF16, kind="ExternalInput")
    bias8 = nc.dram_tensor(
        "bias8", [C_OUT, G, V], F32, kind="ExternalInput"
    )
    out = nc.dram_tensor(
        "out", [N_PER_CORE, C_OUT, T, V], F32, kind="ExternalOutput"
    )

    with TileContext(nc) as tc:
        with (
            tc.tile_pool(name="const", bufs=1) as cpool,
            tc.tile_pool(name="xin", bufs=6) as xpool,
            tc.tile_pool(name="yt", bufs=4) as ytpool,
            tc.tile_pool(name="o", bufs=5) as opool,
            tc.tile_pool(name="ps_y", bufs=5, space="PSUM") as ps_y,
            tc.tile_pool(name="ps_o", bufs=3, space="PSUM") as ps_o,
        ):
            # consts on the gpsimd DMA queue so the sync queue's first x-tile
            # descriptor issues immediately
            wt_sb = cpool.tile([C_IN, K * C_OUT], F32R, tag="wt")
            nc.gpsimd.dma_start(out=wt_sb[:], in_=wt[:].bitcast(F32R))
            mak_sb = cpool.tile([128, K, 128], BF16, tag="mak")
            nc.gpsimd.dma_start(out=mak_sb[:], in_=mak[:])
            bias_sb = cpool.tile([C_OUT, G, V], F32, tag="bias")
            nc.gpsimd.dma_start(out=bias_sb[:], in_=bias8[:])

            # Software-pipelined emission: stage 1 of group i runs while
            # stage 2 of group i-1 consumes yT drained during i's stage 1.
            for _ in range(reps):
                groups = [
                    (n, g) for n in range(N_PER_CORE) for g in range(N_GROUPS)
                ]
                st = {}

                def stage1(n, g):
                    x_sb = xpool.tile([C_IN, G * V], F32R, tag="x", name="x_sb")
                    t0 = g * G
                    nc.sync.dma_start(
                        out=x_sb[:],
                        in_=xs[n, :, t0 : t0 + G, :].bitcast(F32R),
                    )
                    yt_sb = ytpool.tile(
                        [128, 4, K * C_OUT], BF16, tag="yt", name="yt_sb"
                    )
                    # one 1-bank PSUM tile per pair (5-deep pool: the
                    # drain + its PSUM-ack round-trip is ~2x the pair
                    # cadence, so deep buffering keeps the PE from ever
                    # waiting on a bank); drains alternate DVE/ACT
                    for j in range(4):
                        yt_ps = ps_y.tile([128, 512], F32, tag="ytp")
                        nc.tensor.matmul(
                            yt_ps[:, 0 : K * C_OUT],
                            x_sb[:, j * 128 : (j + 1) * 128],
                            wt_sb[:],
                            start=True,
                            stop=True,
                        )
                        # plain drain-cast f32 -> bf16
                        if j % 2 == 0:
                            nc.vector.tensor_copy(
                                out=yt_sb[:, j, :],
                                in_=yt_ps[:, 0 : K * C_OUT],
                            )
                        else:
                            nc.scalar.copy(
                                out=yt_sb[:, j, :],
                                in_=yt_ps[:, 0 : K * C_OUT],
                            )
                    st[(n, g)] = yt_sb

                def stage2(n, g, tail):
                    yt_sb = st.pop((n, g))
                    o_ps = ps_o.tile(
                        [C_OUT, 4, 2 * V], F32, tag="op", name="o2_ps"
                    )
                    for j in range(4):
                        for k in range(K):
                            nc.tensor.matmul(
                                o_ps[:, j, :],
                                yt_sb[:, j, k * 128 : (k + 1) * 128],
                                mak_sb[:, k, :],
                                start=(k == 0),
                                stop=(k == K - 1),
                                skip_group_check=True,
                            )
                    o_sb2 = opool.tile(
                        [C_OUT, G * V], F32, tag="o2", name="o_sb2"
                    )
                    if tail:
                        # epilogue: no yt drains left to compete with, so
                        # fuse drain+bias on DVE and skip the GpSimd hop
                        # (shortens the post-compute tail)
                        nc.vector.tensor_add(
                            out=o_sb2[:],
                            in0=o_ps[:],
                            in1=bias_sb[:],
                        )
                    else:
                        # plain f32 drain, alternating engine; bias on the
                        # GpSimd engine (SBUF-only)
                        o_sb = opool.tile(
                            [C_OUT, G * V], F32, tag="o", name="o_sb"
                        )
                        if g % 2 == 0:
                            nc.vector.tensor_copy(out=o_sb[:], in_=o_ps[:])
                        else:
                            nc.scalar.copy(out=o_sb[:], in_=o_ps[:])
                        nc.gpsimd.tensor_add(
                            out=o_sb2[:],
                            in0=o_sb[:],
                            in1=bias_sb[:],
                        )
                    # out-DMA descgen on the SP queue (GpSimd is TT-bound)
                    t0 = g * G
                    nc.sync.dma_start(
                        out=out[n, :, t0 : t0 + G, :],
                        in_=o_sb2[:],
                    )

                # stage2 first: its DVE/ACT copies are ready to run, so they
                # must enqueue ahead of stage1's drains (which wait on fresh
                # matmuls) to avoid head-of-line blocking
                for i in range(len(groups) + 2):
                    if i >= 2:
                        stage2(*groups[i - 2], tail=(i - 2 >= len(groups) - 2))
                    if i < len(groups):
                        stage1(*groups[i])

    nc.compile()
    return nc


def prep_weights(A, W, b):
    A = np.asarray(A, np.float32)
    W = np.asarray(W, np.float32)
    b = np.asarray(b, np.float32)
    # wt[ci, (k,c)]
    wt = np.ascontiguousarray(
        W.reshape(K, C_OUT, C_IN).transpose(2, 0, 1).reshape(C_IN, K * C_OUT)
    )
    # mak[(h,v), k, (h',w)] = A[k,v,w] * delta_{h,h'}
    m = np.zeros((2, V, K, 2, V), np.float32)
    for h in range(2):
        m[h, :, :, h, :] = A.transpose(1, 0, 2)
    mak = m.reshape(128, K, 128).astype(ml_dtypes.bfloat16)
    # bias2[c,w] broadcast over the 8 t's of a group output tile
    bias2 = np.einsum("kc,kw->cw", b.reshape(K, C_OUT), A.sum(axis=1))
    bias8 = np.ascontiguousarray(
        np.broadcast_to(bias2[:, None, :], (C_OUT, G, V))
    ).astype(np.float32)
    return wt, mak, bias8


_NC_CACHE = {}


def get_nc(reps: int = 1):
    if reps not in _NC_CACHE:
        _NC_CACHE[reps] = build(reps)
    return _NC_CACHE[reps]


def make_in_maps(x, A, W, b):
    x = np.asarray(x, np.float32)
    wt, mak, bias8 = prep_weights(A, W, b)
    return [
        {
            "xs": np.ascontiguousarray(x[i * N_PER_CORE : (i + 1) * N_PER_CORE]),
            "wt": wt,
            "mak": mak,
            "bias8": bias8,
        }
        for i in range(N_CORES)
    ]


def run(x, A, W, b, reps: int = 1):
    nc = get_nc(reps)
    in_maps = make_in_maps(x, A, W, b)
    res = run_bass_kernel_spmd(nc, in_maps, list(range(N_CORES)))
    return np.concatenate(
        [np.asarray(res.results[i]["out"]) for i in range(N_CORES)], axis=0
    )


def kernel(x, A, W, b):
    return run(x, A, W, b, reps=1)


# revision 18
# speedup vs baseline: 1.0328x; 1.0328x over previous
"""Trainium2 Bass kernel for ConvTemporalGraphical (gnn_message_passing).

Reference computation (fp32):
    y   = einsum('nctv,oc->notv', x, W) + b        # 1x1 conv channel mix
    out = einsum('nkctv,kvw->nctw', y.reshape(n,K,C,t,v), A)

Shapes: x [16,128,256,64] f32, A [3,64,64], W [384,128], b [384].

Strategy (8 NeuronCores, data-parallel over N, 2 samples per core):
  W-contraction first, producing the intermediate TRANSPOSED so no PE
  transposes are needed anywhere:
      stage 1:  yT[(t,v), (k,c)] = sum_ci x[ci,t,v] * Wt[ci,(k,c)]
                (lhsT = x in its natural layout, fp32r, FD=384)
      stage 2:  out[c, (t,w)]   += sum_{v} yT[(t,v),(k,c)] * A[k,v,w]
                (bf16, FD=128, PSUM-accumulated over k; MA_k block-diag
                 over the two t's of a pair to use all 128 partitions)

  Engine budget: PSUM can only be drained by DVE and ACT (not GpSimd,
  not DMA), and only DVE can run TensorTensor. So both PSUM drains are
  PLAIN casts/copies, pinned to alternate between DVE and ACT, and the
  bias2[c,w] = sum_{k,v} b[(k,c)] A[k,v,w] add runs on the otherwise-
  idle GpSimd engine in SBUF (out_sb + bias -> out_sb2) before the
  output DMA. Output copies and the bias add are batched per 2 groups.

  Measured pacing (this hw): FD=384 f32r b2b 195ns/MM, FD=128 bf16 b2b
  62ns/MM -> PE ~1525ns per 8-t group = ~98us/core (the wall). DVE
  ~1280ns/group, ACT ~1125, GpSimd ~1380, SP ~613.

kernel(**inputs) shards on host, runs the SPMD program on cores 0-7, and
concatenates the per-core outputs.
"""

import numpy as np
import ml_dtypes

import concourse.bass as bass
import concourse.mybir as mybir
from concourse import bacc
from concourse.bass_utils import run_bass_kernel_spmd
from concourse.tile import TileContext

F32 = mybir.dt.float32
F32R = mybir.dt.float32r
BF16 = mybir.dt.bfloat16

N, C_IN, C_OUT, K, T, V = 16, 128, 128, 3, 256, 64
N_CORES = 8
N_PER_CORE = N // N_CORES  # 2
G = 8                      # t's per group
N_GROUPS = T // G          # 32 groups per sample


def build(reps: int = 1):
    nc = bacc.Bacc(
        "TRN2", target_bir_lowering=False, debug=False, num_devices=N_CORES
    )
    xs = nc.dram_tensor("xs", [N_PER_CORE, C_IN, T, V], F32, kind="ExternalInput")
    wt = nc.dram_tensor("wt", [C_IN, K * C_OUT], F32, kind="ExternalInput")
    mak = nc.dram_tensor("mak", [128, K, 128], BF16, kind="ExternalInput")
    bias8 = nc.dram_tensor(
        "bias8", [C_OUT, G, V], F32, kind="ExternalInput"
    )
    out = nc.dram_tensor(
        "out", [N_PER_CORE, C_OUT, T, V], F32, kind="ExternalOutput"
    )

    with TileContext(nc) as tc:
        with (
            tc.tile_pool(name="const", bufs=1) as cpool,
            tc.tile_pool(name="xin", bufs=6) as xpool,
            tc.tile_pool(name="yt", bufs=4) as ytpool,
            tc.tile_pool(name="o", bufs=5) as opool,
            tc.tile_pool(name="ps_y", bufs=3, space="PSUM") as ps_y,
            tc.tile_pool(name="ps_o", bufs=2, space="PSUM") as ps_o,
        ):
            # consts on the gpsimd DMA queue so the sync queue's first x-tile
            # descriptor issues immediately
            wt_sb = cpool.tile([C_IN, K * C_OUT], F32R, tag="wt")
            nc.gpsimd.dma_start(out=wt_sb[:], in_=wt[:].bitcast(F32R))
            mak_sb = cpool.tile([128, K, 128], BF16, tag="mak")
            nc.gpsimd.dma_start(out=mak_sb[:], in_=mak[:])
            bias_sb = cpool.tile([C_OUT, G, V], F32, tag="bias")
            nc.gpsimd.dma_start(out=bias_sb[:], in_=bias8[:])

            # Software-pipelined emission: stage 1 of group i runs while
            # stage 2 of group i-1 consumes yT drained during i's stage 1.
            for _ in range(reps):
                groups = [
                    (n, g) for n in range(N_PER_CORE) for g in range(N_GROUPS)
                ]
                st = {}

                def stage1(n, g):
                    x_sb = xpool.tile([C_IN, G * V], F32R, tag="x", name="x_sb")
                    t0 = g * G
                    nc.sync.dma_start(
                        out=x_sb[:],
                        in_=xs[n, :, t0 : t0 + G, :].bitcast(F32R),
                    )
                    yt_sb = ytpool.tile(
                        [128, 4, K * C_OUT], BF16, tag="yt", name="yt_sb"
                    )
                    # 2 pairs per 2-bank PSUM tile; one batched 768-elem
                    # drain per half, pinned to alternating engines
                    for h in range(2):
                        yt_ps = ps_y.tile([128, 2, 512], F32, tag="ytp")
                        for jj in range(2):
                            nc.tensor.matmul(
                                yt_ps[:, jj, 0 : K * C_OUT],
                                x_sb[:, (2 * h + jj) * 128 : (2 * h + jj + 1) * 128],
                                wt_sb[:],
                                start=True,
                                stop=True,
                            )
                        # plain drain-cast f32 -> bf16, pinned per half
                        if h == 0:
                            nc.vector.tensor_copy(
                                out=yt_sb[:, 0:2, :],
                                in_=yt_ps[:, :, 0 : K * C_OUT],
                            )
                        else:
                            nc.scalar.copy(
                                out=yt_sb[:, 2:4, :],
                                in_=yt_ps[:, :, 0 : K * C_OUT],
                            )
                    st[(n, g)] = yt_sb

                def stage2(n, g, tail):
                    yt_sb = st.pop((n, g))
                    o_ps = ps_o.tile(
                        [C_OUT, 4, 2 * V], F32, tag="op", name="o2_ps"
                    )
                    for j in range(4):
                        for k in range(K):
                            nc.tensor.matmul(
                                o_ps[:, j, :],
                                yt_sb[:, j, k * 128 : (k + 1) * 128],
                                mak_sb[:, k, :],
                                start=(k == 0),
                                stop=(k == K - 1),
                                skip_group_check=True,
                            )
                    o_sb2 = opool.tile(
                        [C_OUT, G * V], F32, tag="o2", name="o_sb2"
                    )
                    if tail:
                        # epilogue: no yt drains left to compete with, so
                        # fuse drain+bias on DVE and skip the GpSimd hop
                        # (shortens the post-compute tail)
                        nc.vector.tensor_add(
                            out=o_sb2[:],
                            in0=o_ps[:],
                            in1=bias_sb[:],
                        )
                    else:
                        # plain f32 drain, alternating engine; bias on the
                        # GpSimd engine (SBUF-only)
                        o_sb = opool.tile(
                            [C_OUT, G * V], F32, tag="o", name="o_sb"
                        )
                        if g % 2 == 0:
                            nc.vector.tensor_copy(out=o_sb[:], in_=o_ps[:])
                        else:
                            nc.scalar.copy(out=o_sb[:], in_=o_ps[:])
                        nc.gpsimd.tensor_add(
                            out=o_sb2[:],
                            in0=o_sb[:],
                            in1=bias_sb[:],
                        )
                    # out-DMA descgen on the SP queue (GpSimd is TT-bound)
                    t0 = g * G
                    nc.sync.dma_start(
                        out=out[n, :, t0 : t0 + G, :],
                        in_=o_sb2[:],
                    )

                # stage2 first: its DVE/ACT copies are ready to run, so they
                # must enqueue ahead of stage1's drains (which wait on fresh
                # matmuls) to avoid head-of-line blocking
                for i in range(len(groups) + 2):
                    if i >= 2:
                        stage2(*groups[i - 2], tail=(i - 2 >= len(groups) - 2))
                    if i < len(groups):
                        stage1(*groups[i])

    nc.compile()
    return nc


def prep_weights(A, W, b):
    A = np.asarray(A, np.float32)
    W = np.asarray(W, np.float32)
    b = np.asarray(b, np.float32)
    # wt[ci, (k,c)]
    wt = np.ascontiguousarray(
        W.reshape(K, C_OUT, C_IN).transpose(2, 0, 1).reshape(C_IN, K * C_OUT)
    )
    # mak[(h,v), k, (h',w)] = A[k,v,w] * delta_{h,h'}
    m = np.zeros((2, V, K, 2, V), np.float32)
    for h in range(2):
        m[h, :, :, h, :] = A.transpose(1, 0, 2)
    mak = m.reshape(128, K, 128).astype(ml_dtypes.bfloat16)
    # bias2[c,w] broadcast over the 8 t's of a group output tile
    bias2 = np.einsum("kc,kw->cw", b.reshape(K, C_OUT), A.sum(axis=1))
    bias8 = np.ascontiguousarray(
        np.broadcast_to(bias2[:, None, :], (C_OUT, G, V))
    ).astype(np.float32)
    return wt, mak, bias8


_NC_CACHE = {}


def get_nc(reps: int = 1):
    if reps not in _NC_CACHE:
        _NC_CACHE[reps] = build(reps)
    return _NC_CACHE[reps]


def make_in_maps(x, A, W, b):
    x = np.asarray(x, np.float32)
    wt, mak, bias8 = prep_weights(A, W, b)
    return [
        {
            "xs": np.ascontiguousarray(x[i * N_PER_CORE : (i + 1) * N_PER_CORE]),
            "wt": wt,
            "mak": mak,
            "bias8": bias8,
        }
        for i in range(N_CORES)
    ]


def run(x, A, W, b, reps: int = 1):
    nc = get_nc(reps)
    in_maps = make_in_maps(x, A, W, b)
    res = run_bass_kernel_spmd(nc, in_maps, list(range(N_CORES)))
    return np.concatenate(
        [np.asarray(res.results[i]["out"]) for i in range(N_CORES)], axis=0
    )


def kernel(x, A, W, b):
    return run(x, A, W, b, reps=1)


# revision 19
# speedup vs baseline: 1.2198x; 1.1811x over previous
"""Trainium2 Bass kernel for ConvTemporalGraphical (gnn_message_passing).

Reference computation (fp32):
    y   = einsum('nctv,oc->notv', x, W) + b        # 1x1 conv channel mix
    out = einsum('nkctv,kvw->nctw', y.reshape(n,K,C,t,v), A)

Shapes: x [16,128,256,64] f32, A [3,64,64], W [384,128], b [384].

Strategy (8 NeuronCores, data-parallel over N, 2 samples per core):
  The two contractions are reordered as
      Z_k[ci,t,w] = sum_v x[ci,t,v] * A[k,v,w]          (graph mixing first)
      out[c,t,w]  = sum_k sum_ci W[(k,c),ci] * Z_k[ci,t,w] + bias2[c,w]
  with bias2[c,w] = sum_{k,v} b[(k,c)] A[k,v,w] (host-precomputed).

  On-device per (n, 8-t group):
    1. DMA x tile [ci=128, 8*64] (contiguous 2KB/partition).
    2. PE-transpose per 2-t pair: [ci,128] -> xt [(t0 v|t1 v), ci], fp32r.
       Four transposes share one PSUM bank; the drain converts to bf16
       (rounding x once, before the A-contraction).
    3. Step A matmul (bf16, FD=384): lhsT=xt pair, rhs=MA where MA [128,384]
       is block-diag([Acat, Acat]), Acat[v,(k w)]=A[k,v,w]. The zero blocks
       keep the two t's of a pair independent while using all 128 partitions.
       Two pair-outputs share a 2-bank PSUM tile; the drain converts Z to
       bf16 in a per-group [ci, 8, 3, 64] SBUF buffer.
    4. Step B matmul (bf16, FD=512): accumulate over k in PSUM:
       lhsT=Wt[:,k,:] ([ci,c] bf16), rhs=Z[:, :, k, :] (strided).
    5. Drain with fused bias add (f32) -> out chunk tile -> DMA out per
       32-t chunk.

  bf16 operands get fast PE weight loads and halved SBUF traffic; the
  fp32->bf16 rounding rides the PSUM drains, which have to run anyway.
  Measured ~0.26% relative error (tolerance is 2e-2).

kernel(**inputs) shards on host, runs the SPMD program on cores 0-7, and
concatenates the per-core outputs.
"""

import numpy as np
import ml_dtypes

import concourse.bass as bass
import concourse.mybir as mybir
from concourse import bacc
from concourse.bass_utils import run_bass_kernel_spmd
from concourse.tile import TileContext

F32 = mybir.dt.float32
F32R = mybir.dt.float32r
BF16 = mybir.dt.bfloat16

N, C_IN, C_OUT, K, T, V = 16, 128, 128, 3, 256, 64
N_CORES = 8
N_PER_CORE = N // N_CORES  # 2
TC = 32                    # t-chunk size (out DMA granularity)
N_CHUNKS = T // TC         # 8
QG = TC // 8               # 4 groups (8 t's = 4 pairs) per chunk


def build(reps: int = 1):
    nc = bacc.Bacc(
        "TRN2", target_bir_lowering=False, debug=False, num_devices=N_CORES
    )
    xs = nc.dram_tensor("xs", [N_PER_CORE, C_IN, T, V], F32, kind="ExternalInput")
    wt = nc.dram_tensor("wt", [C_IN, K, C_OUT], BF16, kind="ExternalInput")
    ma = nc.dram_tensor("ma", [128, 2, K, V], BF16, kind="ExternalInput")
    bias2r = nc.dram_tensor("bias2r", [C_OUT, 8, V], F32, kind="ExternalInput")
    ident = nc.dram_tensor("ident", [128, 128], F32, kind="ExternalInput")
    out = nc.dram_tensor(
        "out", [N_PER_CORE, C_OUT, T, V], F32, kind="ExternalOutput"
    )

    with TileContext(nc) as tc:
        with (
            tc.tile_pool(name="const", bufs=1) as cpool,
            tc.tile_pool(name="xin", bufs=8) as xpool,
            tc.tile_pool(name="xt", bufs=3) as xtpool,
            tc.tile_pool(name="z", bufs=3) as zpool,
            tc.tile_pool(name="o", bufs=3) as opool,
            tc.tile_pool(name="ps_xt", bufs=2, space="PSUM") as ps_xt,
            tc.tile_pool(name="ps_z", bufs=2, space="PSUM") as ps_z,
            tc.tile_pool(name="ps_o", bufs=2, space="PSUM") as ps_o,
        ):
            # consts on the gpsimd DMA queue so the sync queue's first x-tile
            # descriptor issues immediately
            ident_sb = cpool.tile([128, 128], F32R, tag="ident")
            nc.gpsimd.dma_start(out=ident_sb[:], in_=ident[:].bitcast(F32R))
            wt_sb = cpool.tile([C_IN, K, C_OUT], BF16, tag="wt")
            nc.gpsimd.dma_start(out=wt_sb[:], in_=wt[:])
            ma_sb = cpool.tile([128, 2, K, V], BF16, tag="ma")
            nc.gpsimd.dma_start(out=ma_sb[:], in_=ma[:])
            bias_sb = cpool.tile([C_OUT, 8, V], F32, tag="bias")
            nc.gpsimd.dma_start(out=bias_sb[:], in_=bias2r[:])

            # Software-pipelined emission: PE's stream is in-order, so a
            # matmul that depends on a same-stage drain stalls the PE for
            # the full DVE/ACT round trip. Emit transposes of group i,
            # step A of group i-1, and step B of group i-2 so every PE op's
            # producer drain has a full group-time to land.
            for _ in range(reps):
                groups = [
                    (n, c, q)
                    for n in range(N_PER_CORE)
                    for c in range(N_CHUNKS)
                    for q in range(QG)
                ]
                st = {}  # (n, c) -> chunk state; (n, c, q) -> group state

                def chunk_state(n, c):
                    if (n, c) not in st:
                        st[(n, c)] = {
                            "o": opool.tile(
                                [C_OUT, TC, V], F32, tag="o", name="o_sb"
                            ),
                        }
                    return st[(n, c)]

                def group_state(n, c, q):
                    if (n, c, q) not in st:
                        st[(n, c, q)] = {
                            "z": zpool.tile(
                                [C_IN, 8, K, V], BF16, tag="z", name="z_sb"
                            ),
                            "xt": None,
                        }
                    return st[(n, c, q)]

                def stage_tp(n, c, q):
                    g = group_state(n, c, q)
                    x_sb = xpool.tile([C_IN, 8 * V], F32R, tag="x", name="x_sb")
                    t0 = c * TC + 8 * q
                    nc.sync.dma_start(
                        out=x_sb[:],
                        in_=xs[n, :, t0 : t0 + 8, :].bitcast(F32R),
                    )
                    # 4 transposes -> one PSUM bank as ONE accumulation group
                    # (start clears the bank, so only the first sets it)
                    xt_ps = ps_xt.tile([128, 4, 128], F32R, tag="xtp")
                    for j in range(4):
                        nc.tensor.matmul(
                            xt_ps[:, j, :],
                            x_sb[:, j * 128 : (j + 1) * 128],
                            ident_sb[:],
                            is_transpose=True,
                            start=(j == 0),
                            stop=(j == 3),
                            skip_group_check=True,
                        )
                    # drain converts fp32r -> bf16
                    xt_sb = xtpool.tile([128, 4, 128], BF16, tag="xt")
                    nc.any.tensor_copy(out=xt_sb[:], in_=xt_ps[:])
                    g["xt"] = xt_sb

                def stage_a(n, c, q):
                    g = group_state(n, c, q)
                    xt_sb = g["xt"]
                    # 2 pair-matmuls into one 2-bank PSUM tile (each matmul
                    # stays inside its own 2KB bank), one batched drain that
                    # converts to bf16
                    for h in range(2):
                        z_ps = ps_z.tile([C_IN, 2, 512], F32, tag="zp")
                        for jj in range(2):
                            nc.tensor.matmul(
                                z_ps[:, jj, 0 : 2 * K * V],
                                xt_sb[:, 2 * h + jj, :],
                                ma_sb[:],
                                start=True,
                                stop=True,
                            )
                        nc.any.tensor_copy(
                            out=g["z"][:, 4 * h : 4 * h + 4, :, :],
                            in_=z_ps[:, :, 0 : 2 * K * V],
                        )

                def stage_b(n, c, q):
                    s = chunk_state(n, c)
                    g = group_state(n, c, q)
                    o_ps = ps_o.tile([C_OUT, 8, V], F32, tag="op")
                    for k in range(K):
                        nc.tensor.matmul(
                            o_ps[:],
                            wt_sb[:, k, :],
                            g["z"][:, :, k, :],
                            start=(k == 0),
                            stop=(k == K - 1),
                        )
                    nc.vector.tensor_add(
                        out=s["o"][:, 8 * q : 8 * (q + 1), :],
                        in0=o_ps[:],
                        in1=bias_sb[:],
                    )
                    del st[(n, c, q)]
                    if q == QG - 1:
                        # separate engine queue from the x-input DMAs so the
                        # in/out streams run on different DMA queues
                        nc.gpsimd.dma_start(
                            out=out[n, :, c * TC : (c + 1) * TC, :],
                            in_=s["o"][:],
                        )
                        del st[(n, c)]

                for i in range(len(groups) + 2):
                    if i < len(groups):
                        stage_tp(*groups[i])
                    if 1 <= i < len(groups) + 1:
                        stage_a(*groups[i - 1])
                    if i >= 2:
                        stage_b(*groups[i - 2])

    nc.compile()
    return nc


def prep_weights(A, W, b):
    A = np.asarray(A, np.float32)
    W = np.asarray(W, np.float32)
    b = np.asarray(b, np.float32)
    wt = np.ascontiguousarray(
        W.reshape(K, C_OUT, C_IN).transpose(2, 0, 1)
    ).astype(ml_dtypes.bfloat16)  # [ci, k, c]
    acat = np.ascontiguousarray(A.transpose(1, 0, 2))  # [v, k, w]
    ma = np.zeros((128, 2, K, V), np.float32)
    ma[0:64, 0] = acat
    ma[64:128, 1] = acat
    ma = ma.astype(ml_dtypes.bfloat16)
    bias2 = np.einsum("kc,kw->cw", b.reshape(K, C_OUT), A.sum(axis=1))
    bias2r = np.ascontiguousarray(
        np.broadcast_to(bias2[:, None, :], (C_OUT, 8, V))
    ).astype(np.float32)
    ident = np.eye(128, dtype=np.float32)
    return wt, ma, bias2r, ident


_NC_CACHE = {}


def get_nc(reps: int = 1):
    if reps not in _NC_CACHE:
        _NC_CACHE[reps] = build(reps)
    return _NC_CACHE[reps]


def make_in_maps(x, A, W, b):
    x = np.asarray(x, np.float32)
    wt, ma, bias2r, ident = prep_weights(A, W, b)
    return [
        {
            "xs": np.ascontiguousarray(x[i * N_PER_CORE : (i + 1) * N_PER_CORE]),
            "wt": wt,
            "ma": ma,
            "bias2r": bias2r,
            "ident": ident,
        }
        for i in range(N_CORES)
    ]


def run(x, A, W, b, reps: int = 1):
    nc = get_nc(reps)
    in_maps = make_in_maps(x, A, W, b)
    res = run_bass_kernel_spmd(nc, in_maps, list(range(N_CORES)))
    return np.concatenate(
        [np.asarray(res.results[i]["out"]) for i in range(N_CORES)], axis=0
    )


def kernel(x, A, W, b):
    return run(x, A, W, b, reps=1)


# revision 20
# speedup vs baseline: 1.2232x; 1.0028x over previous
"""Trainium2 Bass kernel for ConvTemporalGraphical (gnn_message_passing).

Reference computation (fp32):
    y   = einsum('nctv,oc->notv', x, W) + b        # 1x1 conv channel mix
    out = einsum('nkctv,kvw->nctw', y.reshape(n,K,C,t,v), A)

Shapes: x [16,128,256,64] f32, A [3,64,64], W [384,128], b [384].

Strategy (8 NeuronCores, data-parallel over N, 2 samples per core):
  The two contractions are reordered as
      Z_k[ci,t,w] = sum_v x[ci,t,v] * A[k,v,w]          (graph mixing first)
      out[c,t,w]  = sum_k sum_ci W[(k,c),ci] * Z_k[ci,t,w] + bias2[c,w]
  with bias2[c,w] = sum_{k,v} b[(k,c)] A[k,v,w] (host-precomputed).

  On-device per (n, 8-t group):
    1. DMA x tile [ci=128, 8*64] (contiguous 2KB/partition).
    2. PE-transpose per 2-t pair: [ci,128] -> xt [(t0 v|t1 v), ci]. Only
       the HIGH bf16-viewed half of each f32 is transposed (= bf16
       truncation, free rounding): bf16 transposes run at 1.0 cycles/row
       vs fp32r's 1.5, and the bf16->bf16 drain is DVE 2x-eligible.
       Four transposes share one PSUM bank.
    3. Step A matmul (bf16, FD=384): lhsT=xt pair, rhs=MA where MA [128,384]
       is block-diag([Acat, Acat]), Acat[v,(k w)]=A[k,v,w]. The zero blocks
       keep the two t's of a pair independent while using all 128 partitions.
       Two pair-outputs share a 2-bank PSUM tile; the drain converts Z to
       bf16 in a per-group [ci, 8, 3, 64] SBUF buffer.
    4. Step B matmul (bf16, FD=512): accumulate over k in PSUM:
       lhsT=Wt[:,k,:] ([ci,c] bf16), rhs=Z[:, :, k, :] (strided).
    5. Drain with fused bias add (f32) -> out chunk tile -> DMA out per
       32-t chunk.

  bf16 operands get fast PE weight loads and halved SBUF traffic; the
  fp32->bf16 rounding rides the PSUM drains, which have to run anyway.
  Measured ~0.26% relative error (tolerance is 2e-2).

kernel(**inputs) shards on host, runs the SPMD program on cores 0-7, and
concatenates the per-core outputs.
"""

import numpy as np
import ml_dtypes

import concourse.bass as bass
import concourse.mybir as mybir
from concourse import bacc
from concourse.bass_utils import run_bass_kernel_spmd
from concourse.tile import TileContext

F32 = mybir.dt.float32
F32R = mybir.dt.float32r
BF16 = mybir.dt.bfloat16

N, C_IN, C_OUT, K, T, V = 16, 128, 128, 3, 256, 64
N_CORES = 8
N_PER_CORE = N // N_CORES  # 2
TC = 32                    # t-chunk size (out DMA granularity)
N_CHUNKS = T // TC         # 8
QG = TC // 8               # 4 groups (8 t's = 4 pairs) per chunk


def build(reps: int = 1):
    nc = bacc.Bacc(
        "TRN2", target_bir_lowering=False, debug=False, num_devices=N_CORES
    )
    xs = nc.dram_tensor("xs", [N_PER_CORE, C_IN, T, V], F32, kind="ExternalInput")
    wt = nc.dram_tensor("wt", [C_IN, K, C_OUT], BF16, kind="ExternalInput")
    ma = nc.dram_tensor("ma", [128, 2, K, V], BF16, kind="ExternalInput")
    bias2r = nc.dram_tensor("bias2r", [C_OUT, 8, V], F32, kind="ExternalInput")
    ident = nc.dram_tensor("ident", [128, 128], BF16, kind="ExternalInput")
    out = nc.dram_tensor(
        "out", [N_PER_CORE, C_OUT, T, V], F32, kind="ExternalOutput"
    )

    with TileContext(nc) as tc:
        with (
            tc.tile_pool(name="const", bufs=1) as cpool,
            tc.tile_pool(name="xin", bufs=8) as xpool,
            tc.tile_pool(name="xt", bufs=3) as xtpool,
            tc.tile_pool(name="z", bufs=3) as zpool,
            tc.tile_pool(name="o", bufs=3) as opool,
            tc.tile_pool(name="ps_xt", bufs=2, space="PSUM") as ps_xt,
            tc.tile_pool(name="ps_z", bufs=2, space="PSUM") as ps_z,
            tc.tile_pool(name="ps_o", bufs=2, space="PSUM") as ps_o,
        ):
            # consts on the gpsimd DMA queue so the sync queue's first x-tile
            # descriptor issues immediately
            ident_sb = cpool.tile([128, 128], BF16, tag="ident")
            nc.gpsimd.dma_start(out=ident_sb[:], in_=ident[:])
            wt_sb = cpool.tile([C_IN, K, C_OUT], BF16, tag="wt")
            nc.gpsimd.dma_start(out=wt_sb[:], in_=wt[:])
            ma_sb = cpool.tile([128, 2, K, V], BF16, tag="ma")
            nc.gpsimd.dma_start(out=ma_sb[:], in_=ma[:])
            bias_sb = cpool.tile([C_OUT, 8, V], F32, tag="bias")
            nc.gpsimd.dma_start(out=bias_sb[:], in_=bias2r[:])

            # Software-pipelined emission: PE's stream is in-order, so a
            # matmul that depends on a same-stage drain stalls the PE for
            # the full DVE/ACT round trip. Emit transposes of group i,
            # step A of group i-1, and step B of group i-2 so every PE op's
            # producer drain has a full group-time to land.
            for _ in range(reps):
                groups = [
                    (n, c, q)
                    for n in range(N_PER_CORE)
                    for c in range(N_CHUNKS)
                    for q in range(QG)
                ]
                st = {}  # (n, c) -> chunk state; (n, c, q) -> group state

                def chunk_state(n, c):
                    if (n, c) not in st:
                        st[(n, c)] = {
                            "o": opool.tile(
                                [C_OUT, TC, V], F32, tag="o", name="o_sb"
                            ),
                        }
                    return st[(n, c)]

                def group_state(n, c, q):
                    if (n, c, q) not in st:
                        st[(n, c, q)] = {
                            "z": zpool.tile(
                                [C_IN, 8, K, V], BF16, tag="z", name="z_sb"
                            ),
                            "xt": None,
                        }
                    return st[(n, c, q)]

                def stage_tp(n, c, q):
                    g = group_state(n, c, q)
                    # x viewed as bf16 pairs: [..., 1] = high half = bf16
                    # truncation of the f32
                    x_sb = xpool.tile([C_IN, 8 * V, 2], BF16, tag="x", name="x_sb")
                    t0 = c * TC + 8 * q
                    nc.sync.dma_start(
                        out=x_sb[:],
                        in_=xs[n, :, t0 : t0 + 8, :].bitcast(BF16),
                    )
                    # 4 transposes of the hi-bf16 planes -> one PSUM bank as
                    # ONE accumulation group (start clears the bank, so only
                    # the first sets it)
                    xt_ps = ps_xt.tile([128, 4, 128], BF16, tag="xtp")
                    for j in range(4):
                        nc.tensor.matmul(
                            xt_ps[:, j, :],
                            x_sb[:, j * 128 : (j + 1) * 128, 1],
                            ident_sb[:],
                            is_transpose=True,
                            start=(j == 0),
                            stop=(j == 3),
                            skip_group_check=True,
                        )
                    # bf16 -> bf16 drain (2x-eligible: all operands 2-byte)
                    xt_sb = xtpool.tile([128, 4, 128], BF16, tag="xt")
                    nc.any.tensor_copy(out=xt_sb[:], in_=xt_ps[:])
                    g["xt"] = xt_sb

                def stage_a(n, c, q):
                    g = group_state(n, c, q)
                    xt_sb = g["xt"]
                    # 2 pair-matmuls into one 2-bank PSUM tile (each matmul
                    # stays inside its own 2KB bank), one batched drain that
                    # converts to bf16
                    for h in range(2):
                        z_ps = ps_z.tile([C_IN, 2, 512], F32, tag="zp")
                        for jj in range(2):
                            nc.tensor.matmul(
                                z_ps[:, jj, 0 : 2 * K * V],
                                xt_sb[:, 2 * h + jj, :],
                                ma_sb[:],
                                start=True,
                                stop=True,
                            )
                        nc.any.tensor_copy(
                            out=g["z"][:, 4 * h : 4 * h + 4, :, :],
                            in_=z_ps[:, :, 0 : 2 * K * V],
                        )

                def stage_b(n, c, q):
                    s = chunk_state(n, c)
                    g = group_state(n, c, q)
                    o_ps = ps_o.tile([C_OUT, 8, V], F32, tag="op")
                    for k in range(K):
                        nc.tensor.matmul(
                            o_ps[:],
                            wt_sb[:, k, :],
                            g["z"][:, :, k, :],
                            start=(k == 0),
                            stop=(k == K - 1),
                        )
                    nc.vector.tensor_add(
                        out=s["o"][:, 8 * q : 8 * (q + 1), :],
                        in0=o_ps[:],
                        in1=bias_sb[:],
                    )
                    del st[(n, c, q)]
                    if q == QG - 1:
                        # separate engine queue from the x-input DMAs so the
                        # in/out streams run on different DMA queues
                        nc.gpsimd.dma_start(
                            out=out[n, :, c * TC : (c + 1) * TC, :],
                            in_=s["o"][:],
                        )
                        del st[(n, c)]

                for i in range(len(groups) + 2):
                    if i < len(groups):
                        stage_tp(*groups[i])
                    if 1 <= i < len(groups) + 1:
                        stage_a(*groups[i - 1])
                    if i >= 2:
                        stage_b(*groups[i - 2])

    nc.compile()
    return nc


def prep_weights(A, W, b):
    A = np.asarray(A, np.float32)
    W = np.asarray(W, np.float32)
    b = np.asarray(b, np.float32)
    wt = np.ascontiguousarray(
        W.reshape(K, C_OUT, C_IN).transpose(2, 0, 1)
    ).astype(ml_dtypes.bfloat16)  # [ci, k, c]
    acat = np.ascontiguousarray(A.transpose(1, 0, 2))  # [v, k, w]
    ma = np.zeros((128, 2, K, V), np.float32)
    ma[0:64, 0] = acat
    ma[64:128, 1] = acat
    ma = ma.astype(ml_dtypes.bfloat16)
    bias2 = np.einsum("kc,kw->cw", b.reshape(K, C_OUT), A.sum(axis=1))
    bias2r = np.ascontiguousarray(
        np.broadcast_to(bias2[:, None, :], (C_OUT, 8, V))
    ).astype(np.float32)
    ident = np.eye(128, dtype=np.float32).astype(ml_dtypes.bfloat16)
    return wt, ma, bias2r, ident


_NC_CACHE = {}


def get_nc(reps: int = 1):
    if reps not in _NC_CACHE:
        _NC_CACHE[reps] = build(reps)
    return _NC_CACHE[reps]


def make_in_maps(x, A, W, b):
    x = np.asarray(x, np.float32)
    wt, ma, bias2r, ident = prep_weights(A, W, b)
    return [
        {
            "xs": np.ascontiguousarray(x[i * N_PER_CORE : (i + 1) * N_PER_CORE]),
            "wt": wt,
            "ma": ma,
            "bias2r": bias2r,
            "ident": ident,
        }
        for i in range(N_CORES)
    ]


def run(x, A, W, b, reps: int = 1):
    nc = get_nc(reps)
    in_maps = make_in_maps(x, A, W, b)
    res = run_bass_kernel_spmd(nc, in_maps, list(range(N_CORES)))
    return np.concatenate(
        [np.asarray(res.results[i]["out"]) for i in range(N_CORES)], axis=0
    )


def kernel(x, A, W, b):
    return run(x, A, W, b, reps=1)


# revision 22
# speedup vs baseline: 1.2286x; 1.0044x over previous
"""Trainium2 Bass kernel for ConvTemporalGraphical (gnn_message_passing).

Reference computation (fp32):
    y   = einsum('nctv,oc->notv', x, W) + b        # 1x1 conv channel mix
    out = einsum('nkctv,kvw->nctw', y.reshape(n,K,C,t,v), A)

Shapes: x [16,128,256,64] f32, A [3,64,64], W [384,128], b [384].

Strategy (8 NeuronCores, data-parallel over N, 2 samples per core):
  The two contractions are reordered as
      Z_k[ci,t,w] = sum_v x[ci,t,v] * A[k,v,w]          (graph mixing first)
      out[c,t,w]  = sum_k sum_ci W[(k,c),ci] * Z_k[ci,t,w] + bias2[c,w]
  with bias2[c,w] = sum_{k,v} b[(k,c)] A[k,v,w] (host-precomputed).

  On-device per (n, 8-t group):
    1. DMA x tile [ci=128, 8*64] (contiguous 2KB/partition).
    2. PE-transpose per 2-t pair: [ci,128] -> xt [(t0 v|t1 v), ci], fp32r.
       Four transposes share one PSUM bank; the drain converts to bf16
       (rounding x once, before the A-contraction).
    3. Step A matmul (bf16, FD=384): lhsT=xt pair, rhs=MA where MA [128,384]
       is block-diag([Acat, Acat]), Acat[v,(k w)]=A[k,v,w]. The zero blocks
       keep the two t's of a pair independent while using all 128 partitions.
       Two pair-outputs share a 2-bank PSUM tile; the drain converts Z to
       bf16 in a per-group [ci, 8, 3, 64] SBUF buffer.
    4. Step B matmul (bf16, FD=512): accumulate over k in PSUM:
       lhsT=Wt[:,k,:] ([ci,c] bf16), rhs=Z[:, :, k, :] (strided).
    5. Drain with fused bias add (f32) -> out chunk tile -> DMA out per
       32-t chunk.

  bf16 operands get fast PE weight loads and halved SBUF traffic; the
  fp32->bf16 rounding rides the PSUM drains, which have to run anyway.
  Measured ~0.26% relative error (tolerance is 2e-2).

kernel(**inputs) shards on host, runs the SPMD program on cores 0-7, and
concatenates the per-core outputs.
"""

import numpy as np
import ml_dtypes

import concourse.bass as bass
import concourse.mybir as mybir
from concourse import bacc
from concourse.bass_utils import run_bass_kernel_spmd
from concourse.tile import TileContext

F32 = mybir.dt.float32
F32R = mybir.dt.float32r
BF16 = mybir.dt.bfloat16

N, C_IN, C_OUT, K, T, V = 16, 128, 128, 3, 256, 64
N_CORES = 8
N_PER_CORE = N // N_CORES  # 2
TC = 32                    # t-chunk size (out DMA granularity)
N_CHUNKS = T // TC         # 8
QG = TC // 8               # 4 groups (8 t's = 4 pairs) per chunk


def build(reps: int = 1):
    nc = bacc.Bacc(
        "TRN2", target_bir_lowering=False, debug=False, num_devices=N_CORES
    )
    xs = nc.dram_tensor("xs", [N_PER_CORE, C_IN, T, V], F32, kind="ExternalInput")
    wt = nc.dram_tensor("wt", [C_IN, K, C_OUT], BF16, kind="ExternalInput")
    ma = nc.dram_tensor("ma", [128, 2, K, V], BF16, kind="ExternalInput")
    bias2r = nc.dram_tensor("bias2r", [C_OUT, 8, V], F32, kind="ExternalInput")
    ident = nc.dram_tensor("ident", [128, 128], F32, kind="ExternalInput")
    out = nc.dram_tensor(
        "out", [N_PER_CORE, C_OUT, T, V], F32, kind="ExternalOutput"
    )

    with TileContext(nc) as tc:
        with (
            tc.tile_pool(name="const", bufs=1) as cpool,
            tc.tile_pool(name="xin", bufs=8) as xpool,
            tc.tile_pool(name="xt", bufs=3) as xtpool,
            tc.tile_pool(name="z", bufs=3) as zpool,
            tc.tile_pool(name="o", bufs=3) as opool,
            tc.tile_pool(name="ps_xt", bufs=2, space="PSUM") as ps_xt,
            tc.tile_pool(name="ps_z", bufs=2, space="PSUM") as ps_z,
            tc.tile_pool(name="ps_o", bufs=2, space="PSUM") as ps_o,
        ):
            # consts on the gpsimd DMA queue so the sync queue's first x-tile
            # descriptor issues immediately
            ident_sb = cpool.tile([128, 128], F32R, tag="ident")
            nc.gpsimd.dma_start(out=ident_sb[:], in_=ident[:].bitcast(F32R))
            ma_sb = cpool.tile([128, 2, K, V], BF16, tag="ma")
            nc.gpsimd.dma_start(out=ma_sb[:], in_=ma[:])
            wt_sb = cpool.tile([C_IN, K, C_OUT], BF16, tag="wt")
            nc.gpsimd.dma_start(out=wt_sb[:], in_=wt[:])
            bias_sb = cpool.tile([C_OUT, 8, V], F32, tag="bias")
            nc.gpsimd.dma_start(out=bias_sb[:], in_=bias2r[:])

            # Software-pipelined emission: PE's stream is in-order, so a
            # matmul that depends on a same-stage drain stalls the PE for
            # the full DVE/ACT round trip. Emit transposes of group i,
            # step A of group i-1, and step B of group i-2 so every PE op's
            # producer drain has a full group-time to land.
            for _ in range(reps):
                groups = [
                    (n, c, q)
                    for n in range(N_PER_CORE)
                    for c in range(N_CHUNKS)
                    for q in range(QG)
                ]
                st = {}  # (n, c) -> chunk state; (n, c, q) -> group state

                def chunk_state(n, c):
                    if (n, c) not in st:
                        st[(n, c)] = {
                            "o": opool.tile(
                                [C_OUT, TC, V], F32, tag="o", name="o_sb"
                            ),
                        }
                    return st[(n, c)]

                def group_state(n, c, q):
                    if (n, c, q) not in st:
                        st[(n, c, q)] = {
                            "z": zpool.tile(
                                [C_IN, 8, K, V], BF16, tag="z", name="z_sb"
                            ),
                            "xt": None,
                        }
                    return st[(n, c, q)]

                def stage_tp(n, c, q):
                    g = group_state(n, c, q)
                    x_sb = xpool.tile([C_IN, 8 * V], F32R, tag="x", name="x_sb")
                    t0 = c * TC + 8 * q
                    nc.sync.dma_start(
                        out=x_sb[:],
                        in_=xs[n, :, t0 : t0 + 8, :].bitcast(F32R),
                    )
                    # 4 transposes -> one PSUM bank as ONE accumulation group
                    # (start clears the bank, so only the first sets it)
                    xt_ps = ps_xt.tile([128, 4, 128], F32R, tag="xtp")
                    for j in range(4):
                        nc.tensor.matmul(
                            xt_ps[:, j, :],
                            x_sb[:, j * 128 : (j + 1) * 128],
                            ident_sb[:],
                            is_transpose=True,
                            start=(j == 0),
                            stop=(j == 3),
                            skip_group_check=True,
                        )
                    # drain converts fp32r -> bf16
                    xt_sb = xtpool.tile([128, 4, 128], BF16, tag="xt")
                    nc.any.tensor_copy(out=xt_sb[:], in_=xt_ps[:])
                    g["xt"] = xt_sb

                def stage_a(n, c, q):
                    g = group_state(n, c, q)
                    xt_sb = g["xt"]
                    # 2 pair-matmuls into one 2-bank PSUM tile (each matmul
                    # stays inside its own 2KB bank), one batched drain that
                    # converts to bf16
                    for h in range(2):
                        z_ps = ps_z.tile([C_IN, 2, 512], F32, tag="zp")
                        for jj in range(2):
                            nc.tensor.matmul(
                                z_ps[:, jj, 0 : 2 * K * V],
                                xt_sb[:, 2 * h + jj, :],
                                ma_sb[:],
                                start=True,
                                stop=True,
                            )
                        nc.any.tensor_copy(
                            out=g["z"][:, 4 * h : 4 * h + 4, :, :],
                            in_=z_ps[:, :, 0 : 2 * K * V],
                        )

                def stage_b(n, c, q):
                    s = chunk_state(n, c)
                    g = group_state(n, c, q)
                    o_ps = ps_o.tile([C_OUT, 8, V], F32, tag="op")
                    for k in range(K):
                        nc.tensor.matmul(
                            o_ps[:],
                            wt_sb[:, k, :],
                            g["z"][:, :, k, :],
                            start=(k == 0),
                            stop=(k == K - 1),
                        )
                    nc.vector.tensor_add(
                        out=s["o"][:, 8 * q : 8 * (q + 1), :],
                        in0=o_ps[:],
                        in1=bias_sb[:],
                    )
                    del st[(n, c, q)]
                    last_chunk = n == N_PER_CORE - 1 and c == N_CHUNKS - 1
                    if last_chunk:
                        # epilogue: stream each group out as soon as its bias
                        # add lands, so the final post-compute DMA is 2KB/
                        # partition instead of 8KB
                        t0 = c * TC + 8 * q
                        nc.gpsimd.dma_start(
                            out=out[n, :, t0 : t0 + 8, :],
                            in_=s["o"][:, 8 * q : 8 * (q + 1), :],
                        )
                        if q == QG - 1:
                            del st[(n, c)]
                    elif q == QG - 1:
                        # separate engine queue from the x-input DMAs so the
                        # in/out streams run on different DMA queues
                        nc.gpsimd.dma_start(
                            out=out[n, :, c * TC : (c + 1) * TC, :],
                            in_=s["o"][:],
                        )
                        del st[(n, c)]

                for i in range(len(groups) + 2):
                    if i < len(groups):
                        stage_tp(*groups[i])
                    if 1 <= i < len(groups) + 1:
                        stage_a(*groups[i - 1])
                    if i >= 2:
                        stage_b(*groups[i - 2])

    nc.compile()
    return nc


def prep_weights(A, W, b):
    A = np.asarray(A, np.float32)
    W = np.asarray(W, np.float32)
    b = np.asarray(b, np.float32)
    wt = np.ascontiguousarray(
        W.reshape(K, C_OUT, C_IN).transpose(2, 0, 1)
    ).astype(ml_dtypes.bfloat16)  # [ci, k, c]
    acat = np.ascontiguousarray(A.transpose(1, 0, 2))  # [v, k, w]
    ma = np.zeros((128, 2, K, V), np.float32)
    ma[0:64, 0] = acat
    ma[64:128, 1] = acat
    ma = ma.astype(ml_dtypes.bfloat16)
    bias2 = np.einsum("kc,kw->cw", b.reshape(K, C_OUT), A.sum(axis=1))
    bias2r = np.ascontiguousarray(
        np.broadcast_to(bias2[:, None, :], (C_OUT, 8, V))
    ).astype(np.float32)
    ident = np.eye(128, dtype=np.float32)
    return wt, ma, bias2r, ident


_NC_CACHE = {}


def get_nc(reps: int = 1):
    if reps not in _NC_CACHE:
        _NC_CACHE[reps] = build(reps)
    return _NC_CACHE[reps]


def make_in_maps(x, A, W, b):
    x = np.asarray(x, np.float32)
    wt, ma, bias2r, ident = prep_weights(A, W, b)
    return [
        {
            "xs": np.ascontiguousarray(x[i * N_PER_CORE : (i + 1) * N_PER_CORE]),
            "wt": wt,
            "ma": ma,
            "bias2r": bias2r,
            "ident": ident,
        }
        for i in range(N_CORES)
    ]


def run(x, A, W, b, reps: int = 1):
    nc = get_nc(reps)
    in_maps = make_in_maps(x, A, W, b)
    res = run_bass_kernel_spmd(nc, in_maps, list(range(N_CORES)))
    return np.concatenate(
        [np.asarray(res.results[i]["out"]) for i in range(N_CORES)], axis=0
    )


def kernel(x, A, W, b):
    return run(x, A, W, b, reps=1)
